# revision 26
# baseline (speedup 1.0000x reference)
"""Multi-head attention (B=8, S=1024, D=2048, H=16) on 8 Trainium2 NeuronCores.

v2 design (all-bf16 matmuls at the PE's ~77 TF/s peak):
  - Pure data parallel: core b computes batch element b; weights replicated.
  - Inputs are pre-transposed and pre-quantized to bf16 on the HOST:
    qT/kT/vT [128p, 16kc, 1024s], W [16h, 128p, 16kc, 128c] — the device
    does ZERO PE transposes.
  - V projection runs in "natural" orientation (stat = vT slice, moving = Wv)
    producing V[s, d] tiles directly; stored in SBUF with a ones-column per
    head so the PV matmul computes softmax denominators for free.
  - Scores in [sk, sq] orientation; exp on ACT -> e bf16.
  - PV-natural: stat = e slice [128sk, 128sq], moving = V|ones [128sk, 129]
    -> out [sq, 128dh | denom]. No sum matmuls, no tail transposes.
  - Software pipeline: each "stretch" interleaves scores(h), pv(h-1) and
    k-proj(h+1) chunkwise so ACT exp latency never stalls the PE.

Self-contained: builds the Bass program, shards inputs, runs SPMD via PJRT,
reassembles the full output.
"""
import numpy as np
from contextlib import ExitStack

import concourse.bacc as bacc
import concourse.mybir as mybir
import concourse.tile as tile

B, S, D, H = 8, 1024, 2048, 16
DH = D // H            # 128
NK = D // 128          # 16 k-chunks
NS = S // 128          # 8 s-tiles
F32 = mybir.dt.float32
BF16 = mybir.dt.bfloat16
SCALE = 1.0 / float(np.sqrt(DH))

_CACHE = {}


def build(opt=None, reps=1, timing=False):
    _defaults = dict(
        ps1024_bufs=3,
        ps129_bufs=2,
        e_bufs=3,          # head-sets of e tiles
        w_bufs=3,
        kh_bufs=2,
        warm_mms=72,       # HAM pre-warm dummy matmuls
        xparts=8,          # qT/vT/kT load split granularity
    )
    _defaults.update(opt or {})
    opt = _defaults
    nc = bacc.Bacc("TRN2", target_bir_lowering=False, debug=False)

    def _in(name, shape, dt_):
        if timing:
            return nc.dram_tensor(name, shape, dt_).ap()
        return nc.dram_tensor(name, shape, dt_, kind="ExternalInput").ap()

    qT_d = _in("qT", [128, NK, S], BF16)
    kT_d = _in("kT", [128, NK, S], BF16)
    vT_d = _in("vT", [128, NK, S], BF16)
    wq_d = _in("Wq", [H, 128, NK, DH], BF16)
    wk_d = _in("Wk", [H, 128, NK, DH], BF16)
    wv_d = _in("Wv", [8, 128, NK, 256], BF16)
    if timing:
        out_d = nc.dram_tensor("out", [S, D], F32).ap()
        tout_d = nc.dram_tensor("tout", [1, 8], F32, kind="ExternalOutput").ap()
    else:
        out_d = nc.dram_tensor("out", [S, D], F32, kind="ExternalOutput").ap()

    with tile.TileContext(nc) as tc, ExitStack() as ctx:
        if timing:
            # zero-fill internal inputs once so exp() stays finite
            with tc.tile_pool(name="zfill", bufs=1) as zpool:
                zf = zpool.tile([128, NK * S], BF16, name="zfill")
                nc.gpsimd.memset(zf[:], 0.0)
                for x in (qT_d, kT_d, vT_d):
                    nc.sync.dma_start(x.rearrange("p a b -> p (a b)"),
                                      zf[:])
                for w in (wq_d, wk_d):
                    for h in range(H):
                        nc.sync.dma_start(
                            w[h].rearrange("p a b -> p (a b)"),
                            zf[:, 0:NK * DH])
                for nb in range(8):
                    nc.sync.dma_start(
                        wv_d[nb].rearrange("p a b -> p (a b)"),
                        zf[:, 0:NK * 256])
        for _rep in range(reps):
            _body_once(nc, tc, qT_d, kT_d, vT_d, wq_d, wk_d, wv_d, out_d, opt)
        if timing:
            with tc.tile_pool(name="zo", bufs=1) as zpool:
                zo = zpool.tile([1, 8], F32, name="zo")
                nc.gpsimd.memset(zo[:], 0.0)
                nc.sync.dma_start(tout_d[:], zo[:])

    nc.compile()
    return nc


def _body_once(nc, tc, qT_d, kT_d, vT_d, wq_d, wk_d, wv_d, out_d, opt):
    with ExitStack() as ctx:
        # --- HAM pre-warm: the first ~10us are DMA-bound (qT streaming in),
        # so the PE would sit idle and start cold (1.2 GHz) when real work
        # arrives.  Dummy matmuls on a memset tile keep the PE-activity
        # monitor busy through the window so head 0 runs at 2.4 GHz.
        with tc.tile_pool(name="warm", bufs=1) as wpool, \
                tc.tile_pool(name="warmp", bufs=1, space="PSUM") as wppool:
            z = wpool.tile([128, 128], BF16, name="warmz")
            wps = wppool.tile([128, 128], F32, name="warmps")
            nc.gpsimd.memset(z[:], 0.0)
            for _ in range(opt["warm_mms"]):
                nc.tensor.matmul(wps[:], z[:], z[:], start=True, stop=True)

        psum = ctx.enter_context(
            tc.tile_pool(name="psum", bufs=opt["ps1024_bufs"], space="PSUM"))
        persist = ctx.enter_context(tc.tile_pool(name="persist", bufs=1))
        # V with ones column: per s-chunk, [128s, H, DH+1]
        v16 = [persist.tile([128, H, DH + 1], BF16, name=f"v16_{m}")
               for m in range(NS)]
        qh_sb = [persist.tile([128, S], BF16, name=f"qh{h}") for h in range(H)]

        def load_xT(pool, name, src, ramp=False):
            t = pool.tile([128, NK, S], BF16, name=name)
            base = 0
            if ramp:
                # finest granularity first: the opening matmuls can start
                # as soon as a half-chunk lands
                for kc in range(2):
                    for half in range(2):
                        nc.sync.dma_start(
                            t[:, kc, half * 512:(half + 1) * 512],
                            src[:, kc, half * 512:(half + 1) * 512])
                base = 2
                widths = [2, 4, 4, 4]
            else:
                widths = [4] * 4
            for w in widths:
                nc.sync.dma_start(t[:, base:base + w, :],
                                  src[:, base:base + w, :])
                base += w
            return t

        # kT lives across phases; wk/kh pools are created early (before
        # Q/V scopes) so their zones don't overlap freed Q/V zones and
        # the Wk[0] DMA can prefetch during phase V.
        pool_xk = ctx.enter_context(tc.tile_pool(name="xTk", bufs=1))
        pool_wk = ctx.enter_context(tc.tile_pool(name="wk", bufs=opt["w_bufs"]))
        pool_kh = ctx.enter_context(tc.tile_pool(name="kh", bufs=opt["kh_bufs"]))

        def load_w(pool, src, eng, parts=4):
            w_t = pool.tile([128, NK, DH], BF16, name="wslice")
            w = NK // parts
            for qtr in range(parts):
                eng.dma_start(w_t[:, qtr * w:(qtr + 1) * w, :],
                              src[:, qtr * w:(qtr + 1) * w, :])
            return w_t

        with ExitStack() as sv:
            pool_wv = sv.enter_context(tc.tile_pool(name="wv", bufs=2))
            pool_xv = sv.enter_context(tc.tile_pool(name="xTv", bufs=1))
            wv_tiles = {}

            # ------------- Phase Q: all heads' q projections -------------
            with ExitStack() as pq:
                pool_xq = pq.enter_context(tc.tile_pool(name="xTq", bufs=1))
                pool_w = pq.enter_context(
                    tc.tile_pool(name="wq", bufs=opt["w_bufs"]))
                wq_pre = load_w(pool_w, wq_d[0], nc.scalar, parts=8)
                # sync-ring FIFO gives qT strict priority; vT/wv0 queue
                # behind it (they aren't needed until phase V).
                qT = load_xT(pool_xq, "qT", qT_d, ramp=True)
                vT = load_xT(pool_xv, "vT", vT_d)
                kT = load_xT(pool_xk, "kT", kT_d)
                wv_tiles[0] = pool_wv.tile([128, NK, 256], BF16, name="wv")
                for h2 in range(2):
                    nc.sync.dma_start(
                        wv_tiles[0][:, h2 * NK // 2:(h2 + 1) * NK // 2, :],
                        wv_d[0][:, h2 * NK // 2:(h2 + 1) * NK // 2, :])
                for h in range(H):
                    w_t = (wq_pre if h == 0 else
                           load_w(pool_w, wq_d[h], nc.scalar, parts=2))
                    ps = psum.tile([128, 1024], F32, name="ps1024")
                    # kc-outer so consumption follows the qT DMA chunk order
                    # (head 0 streams while qT is still arriving from HBM)
                    for kc in range(NK):
                        for half in range(2):
                            nc.tensor.matmul(
                                ps[:, half * 512:(half + 1) * 512],
                                w_t[:, kc, :],
                                qT[:, kc, half * 512:(half + 1) * 512],
                                start=(kc == 0), stop=(kc == NK - 1))
                    for half in range(2):
                        nc.vector.tensor_copy(
                            qh_sb[h][:, half * 512:(half + 1) * 512],
                            ps[:, half * 512:(half + 1) * 512])

            # ------------- Phase V: V = v @ Wv (natural layout) -----------
            for m in range(NS):
                nc.gpsimd.memset(v16[m][:, :, DH:DH + 1], 1.0)
            # prefetch Wk[0] during phase V (sync queue idle by now)
            wk0 = load_w(pool_wk, wk_d[0], nc.sync)
            for nb in range(8):
                if nb in wv_tiles:
                    wv_t = wv_tiles[nb]
                else:
                    wv_t = pool_wv.tile([128, NK, 256], BF16, name="wv")
                    for h2 in range(2):
                        nc.scalar.dma_start(
                            wv_t[:, h2 * NK // 2:(h2 + 1) * NK // 2, :],
                            wv_d[nb][:, h2 * NK // 2:(h2 + 1) * NK // 2, :])
                for m in range(NS):
                    ps = psum.tile([128, 1024], F32, name="ps1024")
                    for kc in range(NK):
                        nc.tensor.matmul(
                            ps[:, 0:256],
                            vT[:, kc, m * 128:(m + 1) * 128],
                            wv_t[:, kc, :],
                            start=(kc == 0), stop=(kc == NK - 1))
                    nc.vector.tensor_copy(
                        v16[m][:, 2 * nb:2 * nb + 2, 0:DH],
                        ps[:, 0:256].rearrange("p (h d) -> p h d", d=DH))

        # -------- Phase K + attention: software-pipelined stretches --------
        with ExitStack() as pk:
            pool_e = pk.enter_context(tc.tile_pool(name="e", bufs=opt["e_bufs"]))
            pool_rs = pk.enter_context(tc.tile_pool(name="rs", bufs=3))
            pool_ot = pk.enter_context(tc.tile_pool(name="ot", bufs=3))
            kstate = {}
            khtile = {}
            kweights = {}

            def kproj_load(h):
                # wk prefetched a full iteration before kproj_begin so the
                # first kproj matmul never waits on the DMA
                kweights[h] = load_w(pool_wk, wk_d[h], nc.sync)

            def kproj_begin(h, w_pre=None):
                w_t = kweights.pop(h) if w_pre is None else w_pre
                ps = psum.tile([128, 1024], F32, name="ps1024")
                khtile[h] = pool_kh.tile([128, S], BF16, name="kh")
                kstate[h] = (w_t, ps)

            def kproj_part(h, c):
                # half-split: kh half-0 is copied out mid-iteration so the
                # next iteration's first scores matmul never waits on it
                w_t, ps = kstate[h]
                half, base = c // 4, (c % 4) * 4
                for kc in range(base, base + 4):
                    nc.tensor.matmul(
                        ps[:, half * 512:(half + 1) * 512],
                        w_t[:, kc, :],
                        kT[:, kc, half * 512:(half + 1) * 512],
                        start=(kc == 0), stop=(kc == NK - 1))
                if base + 4 == NK:
                    nc.vector.tensor_copy(
                        khtile[h][:, half * 512:(half + 1) * 512],
                        ps[:, half * 512:(half + 1) * 512])
                    if half == 1:
                        del kstate[h]

            def scores_chunk(h, e_ts, c):
                ps = psum.tile([128, 1024], F32, name="ps1024")
                for half in range(2):
                    nc.tensor.matmul(
                        ps[:, half * 512:(half + 1) * 512],
                        khtile[h][:, c * 128:(c + 1) * 128],
                        qh_sb[h][:, half * 512:(half + 1) * 512],
                        start=True, stop=True)
                nc.scalar.activation(
                    e_ts[:, c, :], ps[:],
                    mybir.ActivationFunctionType.Exp, scale=SCALE)

            otile = {}

            def pv_tile(h, e_ts, t, epi=False):
                if epi:
                    # epilogue: scores/kproj are done, so borrow the idle
                    # wide-psum pool for a deeper po rotation (no bubbles)
                    po = psum.tile([128, 1024], F32,
                                   name="ps1024")[:, 0:DH + 1]
                else:
                    po = psum.tile([128, DH + 1], F32, name="ps129",
                                   bufs=opt["ps129_bufs"])
                for c in range(NS):
                    nc.tensor.matmul(
                        po[:], e_ts[:, c, t * 128:(t + 1) * 128],
                        v16[c][:, h, :],
                        start=(c == 0), stop=(c == NS - 1))
                rs = pool_rs.tile([128, 1], F32, name="rs")
                nc.vector.reciprocal(rs[:], po[:, DH:DH + 1])
                if t == 0:
                    otile[h] = pool_ot.tile([128, NS, DH], F32, name="ot")
                # normalize on DVE: ACT is the saturated engine in the
                # stretch (exp), and a late exp delays the scores psum-bank
                # rotation; DVE has plenty of slack
                nc.vector.tensor_scalar_mul(otile[h][:, t, :],
                                            po[:, 0:DH], rs[:])
                dst = out_d[:, h * DH:(h + 1) * DH].rearrange(
                    "(t p) d -> p t d", p=128)
                if h >= H - 2:
                    # drain the final heads' output incrementally so the
                    # last DMA after the last matmul is only one t-tile
                    last = (h == H - 1)
                    if t == 3:
                        nc.gpsimd.dma_start(dst[:, 0:4, :],
                                            otile[h][:, 0:4, :])
                    elif t == 5 and last:
                        nc.sync.dma_start(dst[:, 4:6, :],
                                          otile[h][:, 4:6, :])
                    elif t == 6 and last:
                        nc.scalar.dma_start(dst[:, 6:7, :],
                                            otile[h][:, 6:7, :])
                    elif t == NS - 1:
                        if last:
                            nc.sync.dma_start(dst[:, 7:8, :],
                                              otile[h][:, 7:8, :])
                        else:
                            nc.sync.dma_start(dst[:, 4:8, :],
                                              otile[h][:, 4:8, :])
                        del otile[h]
                elif t == NS - 1:
                    (nc.gpsimd if h % 2 == 0 else nc.sync).dma_start(
                        dst, otile[h][:])
                    del otile[h]

            # prologue: kproj(0) densely (weights prefetched during V)
            kproj_begin(0, w_pre=wk0)
            kproj_load(1)
            for c in range(NS):
                kproj_part(0, c)
            # kproj(1) part 0 fills the PE while kh(0)'s half-1 cast
            # drains, so scores(0, 0) never waits at the phase boundary
            kproj_begin(1)
            kproj_load(2)
            kproj_part(1, 0)
            e_prev = None
            for h in range(H):
                e_ts = pool_e.tile([128, NS, S], BF16, name="e")
                if 0 < h < H - 1:
                    kproj_begin(h + 1)
                if 0 < h < H - 2:
                    kproj_load(h + 2)
                for c in range(NS):
                    scores_chunk(h, e_ts, c)
                    if h + 1 < H and not (h == 0 and c == 0):
                        kproj_part(h + 1, c)
                    if e_prev is not None:
                        pv_tile(h - 1, e_prev, c)
                if h - 1 in khtile:
                    del khtile[h - 1]
                e_prev = e_ts
            for t in range(NS):
                pv_tile(H - 1, e_prev, t, epi=True)


def _make_runner(nc, n_cores):
    """Jitted SPMD runner (per-core tensors sharded, weights replicated)."""
    import jax
    from jax.sharding import Mesh, PartitionSpec
    from jax.experimental.shard_map import shard_map
    from concourse import bass2jax
    from concourse.bass2jax import _bass_exec_p, install_neuronx_cc_hook

    install_neuronx_cc_hook()
    partition_name = nc.partition_id_tensor.name if nc.partition_id_tensor else None
    in_names, out_names, out_avals, zero_outs = [], [], [], []
    for alloc in nc.m.functions[0].allocations:
        if not isinstance(alloc, mybir.MemoryLocationSet):
            continue
        name = alloc.memorylocations[0].name
        if alloc.kind == "ExternalInput":
            if name != partition_name:
                in_names.append(name)
        elif alloc.kind == "ExternalOutput":
            out_names.append(name)
            shape = tuple(alloc.tensor_shape)
            dtype = mybir.dt.np(alloc.dtype)
            out_avals.append(jax.core.ShapedArray(shape, dtype))
            zero_outs.append(np.zeros(shape, dtype))
    sharded_in = {"qT", "kT", "vT"}
    in_names_all = in_names + out_names
    if partition_name is not None:
        in_names_all.append(partition_name)

    def _body(*args):
        operands = list(args)
        if partition_name is not None:
            operands.append(bass2jax.partition_id_tensor())
        outs = _bass_exec_p.bind(
            *operands,
            out_avals=tuple(out_avals),
            in_names=tuple(in_names_all),
            out_names=tuple(out_names),
            lowering_input_output_aliases=(),
            sim_require_finite=True,
            sim_require_nnan=True,
            nc=nc,
        )
        return tuple(outs)

    devices = jax.devices()[:n_cores]
    mesh = Mesh(np.asarray(devices), ("core",))
    in_specs = tuple(
        PartitionSpec("core") if n in sharded_in else PartitionSpec()
        for n in in_names
    ) + (PartitionSpec("core"),) * len(out_names)
    out_specs = (PartitionSpec("core"),) * len(out_names)
    jitted = jax.jit(
        shard_map(_body, mesh=mesh, in_specs=in_specs, out_specs=out_specs,
                  check_rep=False),
        keep_unused=True,
    )

    def run(shared_map_, per_core_maps):
        import jax as _jax
        args = []
        for n in in_names:
            if n in sharded_in:
                args.append(np.concatenate([m[n] for m in per_core_maps], axis=0))
            else:
                args.append(shared_map_[n])
        concat_zeros = [
            np.zeros((n_cores * z.shape[0], *z.shape[1:]), z.dtype) for z in zero_outs
        ]
        out_arrs = jitted(*args, *concat_zeros)
        _jax.block_until_ready(out_arrs)
        return [
            {
                name: np.asarray(out_arrs[i]).reshape(n_cores, *out_avals[i].shape)[c]
                for i, name in enumerate(out_names)
            }
            for c in range(n_cores)
        ]

    return run


def _prep_xT(x):
    """[S, D] f32 -> [128, NK, S] bf16 with [p, kc, s] = x[s, kc*128+p]."""
    from ml_dtypes import bfloat16
    xt = np.ascontiguousarray(x.T).reshape(NK, 128, S).transpose(1, 0, 2)
    return np.ascontiguousarray(xt).astype(bfloat16)


def _prep_wqk(w):
    """[D, D] f32 -> [H, 128, NK, DH] bf16, [h,p,kc,c] = w[kc*128+p, h*128+c]."""
    from ml_dtypes import bfloat16
    wr = w.reshape(NK, 128, H, DH).transpose(2, 1, 0, 3)
    return np.ascontiguousarray(wr).astype(bfloat16)


def _prep_wv(w):
    """[D, D] f32 -> [8, 128, NK, 256] bf16, [nb,p,kc,n] = w[kc*128+p, nb*256+n]."""
    from ml_dtypes import bfloat16
    wr = w.reshape(NK, 128, 8, 256).transpose(2, 1, 0, 3)
    return np.ascontiguousarray(wr).astype(bfloat16)


def _get_compiled():
    if "run" not in _CACHE:
        nc = build()
        _CACHE["run"] = _make_runner(nc, B)
    return _CACHE["run"]


def make_input_maps(rng):
    """For profile_run.py: full random per-core input maps."""
    sc = np.float32(1.0 / np.sqrt(D))
    Wq = _prep_wqk(rng.standard_normal((D, D), dtype=np.float32) * sc)
    Wk = _prep_wqk(rng.standard_normal((D, D), dtype=np.float32) * sc)
    Wv = _prep_wv(rng.standard_normal((D, D), dtype=np.float32) * sc)
    maps = []
    for b in range(B):
        maps.append({
            "qT": _prep_xT(rng.standard_normal((S, D), dtype=np.float32)),
            "kT": _prep_xT(rng.standard_normal((S, D), dtype=np.float32)),
            "vT": _prep_xT(rng.standard_normal((S, D), dtype=np.float32)),
            "Wq": Wq, "Wk": Wk, "Wv": Wv,
        })
    return maps


def kernel(q, k, v, Wq, Wk, Wv):
    run = _get_compiled()
    q = np.asarray(q, dtype=np.float32)
    k = np.asarray(k, dtype=np.float32)
    v = np.asarray(v, dtype=np.float32)
    shared = {
        "Wq": _prep_wqk(np.asarray(Wq, dtype=np.float32)),
        "Wk": _prep_wqk(np.asarray(Wk, dtype=np.float32)),
        "Wv": _prep_wv(np.asarray(Wv, dtype=np.float32)),
    }
    per_core = [
        {"qT": _prep_xT(q[b]), "kT": _prep_xT(k[b]), "vT": _prep_xT(v[b])}
        for b in range(B)
    ]
    results = run(shared, per_core)
    out = np.stack([results[b]["out"] for b in range(B)], axis=0)
    return out.astype(np.float32)


if __name__ == "__main__":
    rng = np.random.default_rng(0)
    qq = rng.standard_normal((B, S, D), dtype=np.float32)
    kk = rng.standard_normal((B, S, D), dtype=np.float32)
    vv = rng.standard_normal((B, S, D), dtype=np.float32)
    sc = np.float32(1.0 / np.sqrt(D))
    Wq = rng.standard_normal((D, D), dtype=np.float32) * sc
    Wk = rng.standard_normal((D, D), dtype=np.float32) * sc
    Wv = rng.standard_normal((D, D), dtype=np.float32) * sc
    o = kernel(q=qq, k=kk, v=vv, Wq=Wq, Wk=Wk, Wv=Wv)

    # quick numpy reference check
    qh = (qq.reshape(-1, D) @ Wq).reshape(B, S, H, DH)
    kh = (kk.reshape(-1, D) @ Wk).reshape(B, S, H, DH)
    vh = (vv.reshape(-1, D) @ Wv).reshape(B, S, H, DH)
    scr = np.einsum("bqhd,bkhd->bhqk", qh, kh) * SCALE
    m = scr.max(-1, keepdims=True)
    e = np.exp(scr - m)
    p = e / e.sum(-1, keepdims=True)
    ref = np.einsum("bhqk,bkhd->bqhd", p, vh).reshape(B, S, D)
    err = np.abs(o - ref)
    print("out", o.shape, o.dtype)
    print(f"rel={err.max()/np.abs(ref).max():.4e}")



# revision 29
# speedup vs baseline: 1.0035x; 1.0035x over previous
"""Multi-head attention (B=8, S=1024, D=2048, H=16) on 8 Trainium2 NeuronCores.

v2 design (all-bf16 matmuls at the PE's ~77 TF/s peak):
  - Pure data parallel: core b computes batch element b; weights replicated.
  - Inputs are pre-transposed and pre-quantized to bf16 on the HOST:
    qT/kT/vT [128p, 16kc, 1024s], W [16h, 128p, 16kc, 128c] — the device
    does ZERO PE transposes.
  - V projection runs in "natural" orientation (stat = vT slice, moving = Wv)
    producing V[s, d] tiles directly; stored in SBUF with a ones-column per
    head so the PV matmul computes softmax denominators for free.
  - Scores in [sk, sq] orientation; exp on ACT -> e bf16.
  - PV-natural: stat = e slice [128sk, 128sq], moving = V|ones [128sk, 129]
    -> out [sq, 128dh | denom]. No sum matmuls, no tail transposes.
  - Software pipeline: each "stretch" interleaves scores(h), pv(h-1) and
    k-proj(h+1) chunkwise so ACT exp latency never stalls the PE.

Self-contained: builds the Bass program, shards inputs, runs SPMD via PJRT,
reassembles the full output.
"""
import numpy as np
from contextlib import ExitStack

import concourse.bacc as bacc
import concourse.mybir as mybir
import concourse.tile as tile

B, S, D, H = 8, 1024, 2048, 16
DH = D // H            # 128
NK = D // 128          # 16 k-chunks
NS = S // 128          # 8 s-tiles
F32 = mybir.dt.float32
BF16 = mybir.dt.bfloat16
SCALE = 1.0 / float(np.sqrt(DH))

_CACHE = {}


def build(opt=None, reps=1, timing=False):
    _defaults = dict(
        ps1024_bufs=3,
        ps129_bufs=2,
        e_bufs=3,          # head-sets of e tiles
        w_bufs=3,
        kh_bufs=2,
        xparts=8,          # qT/vT/kT load split granularity
    )
    _defaults.update(opt or {})
    opt = _defaults
    nc = bacc.Bacc("TRN2", target_bir_lowering=False, debug=False)

    def _in(name, shape, dt_):
        if timing:
            return nc.dram_tensor(name, shape, dt_).ap()
        return nc.dram_tensor(name, shape, dt_, kind="ExternalInput").ap()

    qT_d = _in("qT", [128, NK, S], BF16)
    kT_d = _in("kT", [128, NK, S], BF16)
    vT_d = _in("vT", [128, NK, S], BF16)
    wq_d = _in("Wq", [H, 128, NK, DH], BF16)
    wk_d = _in("Wk", [H, 128, NK, DH], BF16)
    wv_d = _in("Wv", [8, 128, NK, 256], BF16)
    if timing:
        out_d = nc.dram_tensor("out", [S, D], F32).ap()
        tout_d = nc.dram_tensor("tout", [1, 8], F32, kind="ExternalOutput").ap()
    else:
        out_d = nc.dram_tensor("out", [S, D], F32, kind="ExternalOutput").ap()

    with tile.TileContext(nc) as tc, ExitStack() as ctx:
        if timing:
            # zero-fill internal inputs once so exp() stays finite
            with tc.tile_pool(name="zfill", bufs=1) as zpool:
                zf = zpool.tile([128, NK * S], BF16, name="zfill")
                nc.gpsimd.memset(zf[:], 0.0)
                for x in (qT_d, kT_d, vT_d):
                    nc.sync.dma_start(x.rearrange("p a b -> p (a b)"),
                                      zf[:])
                for w in (wq_d, wk_d):
                    for h in range(H):
                        nc.sync.dma_start(
                            w[h].rearrange("p a b -> p (a b)"),
                            zf[:, 0:NK * DH])
                for nb in range(8):
                    nc.sync.dma_start(
                        wv_d[nb].rearrange("p a b -> p (a b)"),
                        zf[:, 0:NK * 256])
        for _rep in range(reps):
            _body_once(nc, tc, qT_d, kT_d, vT_d, wq_d, wk_d, wv_d, out_d, opt)
        if timing:
            with tc.tile_pool(name="zo", bufs=1) as zpool:
                zo = zpool.tile([1, 8], F32, name="zo")
                nc.gpsimd.memset(zo[:], 0.0)
                nc.sync.dma_start(tout_d[:], zo[:])

    nc.compile()
    return nc


def _body_once(nc, tc, qT_d, kT_d, vT_d, wq_d, wk_d, wv_d, out_d, opt):
    with ExitStack() as ctx:
        psum = ctx.enter_context(
            tc.tile_pool(name="psum", bufs=opt["ps1024_bufs"], space="PSUM"))
        persist = ctx.enter_context(tc.tile_pool(name="persist", bufs=1))
        # V with ones column: per s-chunk, [128s, H, DH+1]
        v16 = [persist.tile([128, H, DH + 1], BF16, name=f"v16_{m}")
               for m in range(NS)]
        qh_sb = [persist.tile([128, S], BF16, name=f"qh{h}") for h in range(H)]

        def load_xT(pool, name, src, ramp=False):
            t = pool.tile([128, NK, S], BF16, name=name)
            base = 0
            if ramp:
                # finest granularity first: the opening matmuls can start
                # as soon as a half-chunk lands
                for kc in range(2):
                    for half in range(2):
                        nc.sync.dma_start(
                            t[:, kc, half * 512:(half + 1) * 512],
                            src[:, kc, half * 512:(half + 1) * 512])
                base = 2
                widths = [2, 4, 4, 4]
            else:
                widths = [4] * 4
            for w in widths:
                nc.sync.dma_start(t[:, base:base + w, :],
                                  src[:, base:base + w, :])
                base += w
            return t

        # kT lives across phases; wk/kh pools are created early (before
        # Q/V scopes) so their zones don't overlap freed Q/V zones and
        # the Wk[0] DMA can prefetch during phase V.
        pool_xk = ctx.enter_context(tc.tile_pool(name="xTk", bufs=1))
        pool_wk = ctx.enter_context(tc.tile_pool(name="wk", bufs=opt["w_bufs"]))
        pool_kh = ctx.enter_context(tc.tile_pool(name="kh", bufs=opt["kh_bufs"]))

        def load_w(pool, src, eng, parts=4):
            w_t = pool.tile([128, NK, DH], BF16, name="wslice")
            w = NK // parts
            for qtr in range(parts):
                eng.dma_start(w_t[:, qtr * w:(qtr + 1) * w, :],
                              src[:, qtr * w:(qtr + 1) * w, :])
            return w_t

        with ExitStack() as sv:
            pool_wv = sv.enter_context(tc.tile_pool(name="wv", bufs=2))
            pool_xv = sv.enter_context(tc.tile_pool(name="xTv", bufs=1))
            wv_tiles = {}

            # ------------- Phase Q: all heads' q projections -------------
            with ExitStack() as pq:
                pool_xq = pq.enter_context(tc.tile_pool(name="xTq", bufs=1))
                pool_w = pq.enter_context(
                    tc.tile_pool(name="wq", bufs=opt["w_bufs"]))
                wq_pre = load_w(pool_w, wq_d[0], nc.scalar, parts=8)
                # sync-ring FIFO gives qT strict priority; vT/wv0 queue
                # behind it (they aren't needed until phase V).
                qT = load_xT(pool_xq, "qT", qT_d, ramp=True)
                vT = load_xT(pool_xv, "vT", vT_d)
                kT = load_xT(pool_xk, "kT", kT_d)
                wv_tiles[0] = pool_wv.tile([128, NK, 256], BF16, name="wv")
                for h2 in range(2):
                    nc.sync.dma_start(
                        wv_tiles[0][:, h2 * NK // 2:(h2 + 1) * NK // 2, :],
                        wv_d[0][:, h2 * NK // 2:(h2 + 1) * NK // 2, :])
                for h in range(H):
                    w_t = (wq_pre if h == 0 else
                           load_w(pool_w, wq_d[h], nc.scalar, parts=2))
                    ps = psum.tile([128, 1024], F32, name="ps1024")
                    # kc-outer so consumption follows the qT DMA chunk order
                    # (head 0 streams while qT is still arriving from HBM)
                    for kc in range(NK):
                        for half in range(2):
                            nc.tensor.matmul(
                                ps[:, half * 512:(half + 1) * 512],
                                w_t[:, kc, :],
                                qT[:, kc, half * 512:(half + 1) * 512],
                                start=(kc == 0), stop=(kc == NK - 1))
                    for half in range(2):
                        nc.vector.tensor_copy(
                            qh_sb[h][:, half * 512:(half + 1) * 512],
                            ps[:, half * 512:(half + 1) * 512])

            # ------------- Phase V: V = v @ Wv (natural layout) -----------
            for m in range(NS):
                nc.gpsimd.memset(v16[m][:, :, DH:DH + 1], 1.0)
            # prefetch Wk[0] during phase V (sync queue idle by now)
            wk0 = load_w(pool_wk, wk_d[0], nc.sync)
            for nb in range(8):
                if nb in wv_tiles:
                    wv_t = wv_tiles[nb]
                else:
                    wv_t = pool_wv.tile([128, NK, 256], BF16, name="wv")
                    for h2 in range(2):
                        nc.scalar.dma_start(
                            wv_t[:, h2 * NK // 2:(h2 + 1) * NK // 2, :],
                            wv_d[nb][:, h2 * NK // 2:(h2 + 1) * NK // 2, :])
                for m in range(NS):
                    ps = psum.tile([128, 1024], F32, name="ps1024")
                    for kc in range(NK):
                        nc.tensor.matmul(
                            ps[:, 0:256],
                            vT[:, kc, m * 128:(m + 1) * 128],
                            wv_t[:, kc, :],
                            start=(kc == 0), stop=(kc == NK - 1))
                    nc.vector.tensor_copy(
                        v16[m][:, 2 * nb:2 * nb + 2, 0:DH],
                        ps[:, 0:256].rearrange("p (h d) -> p h d", d=DH))

        # -------- Phase K + attention: software-pipelined stretches --------
        with ExitStack() as pk:
            pool_e = pk.enter_context(tc.tile_pool(name="e", bufs=opt["e_bufs"]))
            pool_rs = pk.enter_context(tc.tile_pool(name="rs", bufs=3))
            pool_ot = pk.enter_context(tc.tile_pool(name="ot", bufs=3))
            kstate = {}
            khtile = {}
            kweights = {}

            def kproj_load(h):
                # wk prefetched a full iteration before kproj_begin so the
                # first kproj matmul never waits on the DMA
                kweights[h] = load_w(pool_wk, wk_d[h], nc.sync)

            def kproj_begin(h, w_pre=None):
                w_t = kweights.pop(h) if w_pre is None else w_pre
                ps = psum.tile([128, 1024], F32, name="ps1024")
                khtile[h] = pool_kh.tile([128, S], BF16, name="kh")
                kstate[h] = (w_t, ps)

            def kproj_part(h, c):
                # half-split: kh half-0 is copied out mid-iteration so the
                # next iteration's first scores matmul never waits on it
                w_t, ps = kstate[h]
                half, base = c // 4, (c % 4) * 4
                for kc in range(base, base + 4):
                    nc.tensor.matmul(
                        ps[:, half * 512:(half + 1) * 512],
                        w_t[:, kc, :],
                        kT[:, kc, half * 512:(half + 1) * 512],
                        start=(kc == 0), stop=(kc == NK - 1))
                if base + 4 == NK:
                    nc.vector.tensor_copy(
                        khtile[h][:, half * 512:(half + 1) * 512],
                        ps[:, half * 512:(half + 1) * 512])
                    if half == 1:
                        del kstate[h]

            def scores_chunk(h, e_ts, c):
                ps = psum.tile([128, 1024], F32, name="ps1024")
                for half in range(2):
                    nc.tensor.matmul(
                        ps[:, half * 512:(half + 1) * 512],
                        khtile[h][:, c * 128:(c + 1) * 128],
                        qh_sb[h][:, half * 512:(half + 1) * 512],
                        start=True, stop=True)
                nc.scalar.activation(
                    e_ts[:, c, :], ps[:],
                    mybir.ActivationFunctionType.Exp, scale=SCALE)

            otile = {}

            def pv_tile(h, e_ts, t, epi=False):
                if epi:
                    # epilogue: scores/kproj are done, so borrow the idle
                    # wide-psum pool for a deeper po rotation (no bubbles)
                    po = psum.tile([128, 1024], F32,
                                   name="ps1024")[:, 0:DH + 1]
                else:
                    po = psum.tile([128, DH + 1], F32, name="ps129",
                                   bufs=opt["ps129_bufs"])
                for c in range(NS):
                    nc.tensor.matmul(
                        po[:], e_ts[:, c, t * 128:(t + 1) * 128],
                        v16[c][:, h, :],
                        start=(c == 0), stop=(c == NS - 1))
                rs = pool_rs.tile([128, 1], F32, name="rs")
                nc.vector.reciprocal(rs[:], po[:, DH:DH + 1])
                if t == 0:
                    otile[h] = pool_ot.tile([128, NS, DH], F32, name="ot")
                # normalize on DVE: ACT is the saturated engine in the
                # stretch (exp), and a late exp delays the scores psum-bank
                # rotation; DVE has plenty of slack
                nc.vector.tensor_scalar_mul(otile[h][:, t, :],
                                            po[:, 0:DH], rs[:])
                dst = out_d[:, h * DH:(h + 1) * DH].rearrange(
                    "(t p) d -> p t d", p=128)
                # all out DMAs ride sync (HWDGE): the sync ring is idle in
                # the stretch, and keeping SWDGE (gpsimd) untouched avoids
                # its ~3.5us descriptor-ring DRAIN in the program teardown
                if h >= H - 2:
                    # drain the final heads' output incrementally so the
                    # last DMA after the last matmul is only one t-tile
                    last = (h == H - 1)
                    if t == 3:
                        nc.sync.dma_start(dst[:, 0:4, :],
                                          otile[h][:, 0:4, :])
                    elif t == 5 and last:
                        nc.sync.dma_start(dst[:, 4:6, :],
                                          otile[h][:, 4:6, :])
                    elif t == 6 and last:
                        nc.scalar.dma_start(dst[:, 6:7, :],
                                            otile[h][:, 6:7, :])
                    elif t == NS - 1:
                        if last:
                            nc.sync.dma_start(dst[:, 7:8, :],
                                              otile[h][:, 7:8, :])
                        else:
                            nc.sync.dma_start(dst[:, 4:8, :],
                                              otile[h][:, 4:8, :])
                        del otile[h]
                elif t == NS - 1:
                    nc.sync.dma_start(dst, otile[h][:])
                    del otile[h]

            # prologue: kproj(0) densely (weights prefetched during V)
            kproj_begin(0, w_pre=wk0)
            kproj_load(1)
            for c in range(NS):
                kproj_part(0, c)
            # kproj(1) part 0 fills the PE while kh(0)'s half-1 cast
            # drains, so scores(0, 0) never waits at the phase boundary
            kproj_begin(1)
            kproj_load(2)
            kproj_part(1, 0)
            e_prev = None
            for h in range(H):
                e_ts = pool_e.tile([128, NS, S], BF16, name="e")
                if 0 < h < H - 1:
                    kproj_begin(h + 1)
                if 0 < h < H - 2:
                    kproj_load(h + 2)
                for c in range(NS):
                    scores_chunk(h, e_ts, c)
                    if h + 1 < H and not (h == 0 and c == 0):
                        kproj_part(h + 1, c)
                    if e_prev is not None:
                        pv_tile(h - 1, e_prev, c)
                if h - 1 in khtile:
                    del khtile[h - 1]
                e_prev = e_ts
            for t in range(NS):
                pv_tile(H - 1, e_prev, t, epi=True)


def _make_runner(nc, n_cores):
    """Jitted SPMD runner (per-core tensors sharded, weights replicated)."""
    import jax
    from jax.sharding import Mesh, PartitionSpec
    from jax.experimental.shard_map import shard_map
    from concourse import bass2jax
    from concourse.bass2jax import _bass_exec_p, install_neuronx_cc_hook

    install_neuronx_cc_hook()
    partition_name = nc.partition_id_tensor.name if nc.partition_id_tensor else None
    in_names, out_names, out_avals, zero_outs = [], [], [], []
    for alloc in nc.m.functions[0].allocations:
        if not isinstance(alloc, mybir.MemoryLocationSet):
            continue
        name = alloc.memorylocations[0].name
        if alloc.kind == "ExternalInput":
            if name != partition_name:
                in_names.append(name)
        elif alloc.kind == "ExternalOutput":
            out_names.append(name)
            shape = tuple(alloc.tensor_shape)
            dtype = mybir.dt.np(alloc.dtype)
            out_avals.append(jax.core.ShapedArray(shape, dtype))
            zero_outs.append(np.zeros(shape, dtype))
    sharded_in = {"qT", "kT", "vT"}
    in_names_all = in_names + out_names
    if partition_name is not None:
        in_names_all.append(partition_name)

    def _body(*args):
        operands = list(args)
        if partition_name is not None:
            operands.append(bass2jax.partition_id_tensor())
        outs = _bass_exec_p.bind(
            *operands,
            out_avals=tuple(out_avals),
            in_names=tuple(in_names_all),
            out_names=tuple(out_names),
            lowering_input_output_aliases=(),
            sim_require_finite=True,
            sim_require_nnan=True,
            nc=nc,
        )
        return tuple(outs)

    devices = jax.devices()[:n_cores]
    mesh = Mesh(np.asarray(devices), ("core",))
    in_specs = tuple(
        PartitionSpec("core") if n in sharded_in else PartitionSpec()
        for n in in_names
    ) + (PartitionSpec("core"),) * len(out_names)
    out_specs = (PartitionSpec("core"),) * len(out_names)
    jitted = jax.jit(
        shard_map(_body, mesh=mesh, in_specs=in_specs, out_specs=out_specs,
                  check_rep=False),
        keep_unused=True,
    )

    def run(shared_map_, per_core_maps):
        import jax as _jax
        args = []
        for n in in_names:
            if n in sharded_in:
                args.append(np.concatenate([m[n] for m in per_core_maps], axis=0))
            else:
                args.append(shared_map_[n])
        concat_zeros = [
            np.zeros((n_cores * z.shape[0], *z.shape[1:]), z.dtype) for z in zero_outs
        ]
        out_arrs = jitted(*args, *concat_zeros)
        _jax.block_until_ready(out_arrs)
        return [
            {
                name: np.asarray(out_arrs[i]).reshape(n_cores, *out_avals[i].shape)[c]
                for i, name in enumerate(out_names)
            }
            for c in range(n_cores)
        ]

    return run


def _prep_xT(x):
    """[S, D] f32 -> [128, NK, S] bf16 with [p, kc, s] = x[s, kc*128+p]."""
    from ml_dtypes import bfloat16
    xt = np.ascontiguousarray(x.T).reshape(NK, 128, S).transpose(1, 0, 2)
    return np.ascontiguousarray(xt).astype(bfloat16)


def _prep_wqk(w):
    """[D, D] f32 -> [H, 128, NK, DH] bf16, [h,p,kc,c] = w[kc*128+p, h*128+c]."""
    from ml_dtypes import bfloat16
    wr = w.reshape(NK, 128, H, DH).transpose(2, 1, 0, 3)
    return np.ascontiguousarray(wr).astype(bfloat16)


def _prep_wv(w):
    """[D, D] f32 -> [8, 128, NK, 256] bf16, [nb,p,kc,n] = w[kc*128+p, nb*256+n]."""
    from ml_dtypes import bfloat16
    wr = w.reshape(NK, 128, 8, 256).transpose(2, 1, 0, 3)
    return np.ascontiguousarray(wr).astype(bfloat16)


def _get_compiled():
    if "run" not in _CACHE:
        nc = build()
        _CACHE["run"] = _make_runner(nc, B)
    return _CACHE["run"]


def make_input_maps(rng):
    """For profile_run.py: full random per-core input maps."""
    sc = np.float32(1.0 / np.sqrt(D))
    Wq = _prep_wqk(rng.standard_normal((D, D), dtype=np.float32) * sc)
    Wk = _prep_wqk(rng.standard_normal((D, D), dtype=np.float32) * sc)
    Wv = _prep_wv(rng.standard_normal((D, D), dtype=np.float32) * sc)
    maps = []
    for b in range(B):
        maps.append({
            "qT": _prep_xT(rng.standard_normal((S, D), dtype=np.float32)),
            "kT": _prep_xT(rng.standard_normal((S, D), dtype=np.float32)),
            "vT": _prep_xT(rng.standard_normal((S, D), dtype=np.float32)),
            "Wq": Wq, "Wk": Wk, "Wv": Wv,
        })
    return maps


def kernel(q, k, v, Wq, Wk, Wv):
    run = _get_compiled()
    q = np.asarray(q, dtype=np.float32)
    k = np.asarray(k, dtype=np.float32)
    v = np.asarray(v, dtype=np.float32)
    shared = {
        "Wq": _prep_wqk(np.asarray(Wq, dtype=np.float32)),
        "Wk": _prep_wqk(np.asarray(Wk, dtype=np.float32)),
        "Wv": _prep_wv(np.asarray(Wv, dtype=np.float32)),
    }
    per_core = [
        {"qT": _prep_xT(q[b]), "kT": _prep_xT(k[b]), "vT": _prep_xT(v[b])}
        for b in range(B)
    ]
    results = run(shared, per_core)
    out = np.stack([results[b]["out"] for b in range(B)], axis=0)
    return out.astype(np.float32)


if __name__ == "__main__":
    rng = np.random.default_rng(0)
    qq = rng.standard_normal((B, S, D), dtype=np.float32)
    kk = rng.standard_normal((B, S, D), dtype=np.float32)
    vv = rng.standard_normal((B, S, D), dtype=np.float32)
    sc = np.float32(1.0 / np.sqrt(D))
    Wq = rng.standard_normal((D, D), dtype=np.float32) * sc
    Wk = rng.standard_normal((D, D), dtype=np.float32) * sc
    Wv = rng.standard_normal((D, D), dtype=np.float32) * sc
    o = kernel(q=qq, k=kk, v=vv, Wq=Wq, Wk=Wk, Wv=Wv)

    # quick numpy reference check
    qh = (qq.reshape(-1, D) @ Wq).reshape(B, S, H, DH)
    kh = (kk.reshape(-1, D) @ Wk).reshape(B, S, H, DH)
    vh = (vv.reshape(-1, D) @ Wv).reshape(B, S, H, DH)
    scr = np.einsum("bqhd,bkhd->bhqk", qh, kh) * SCALE
    m = scr.max(-1, keepdims=True)
    e = np.exp(scr - m)
    p = e / e.sum(-1, keepdims=True)
    ref = np.einsum("bhqk,bkhd->bqhd", p, vh).reshape(B, S, D)
    err = np.abs(o - ref)
    print("out", o.shape, o.dtype)
    print(f"rel={err.max()/np.abs(ref).max():.4e}")

